# revision 15
# baseline (speedup 1.0000x reference)
"""Tricubic B-spline grid interpolation (CubicBSplineGrid3d) on 8 Trainium2 cores.

Strategy (data-parallel over queries, per sharding hint):
  * Host: pad grid (64,64,64,32) -> (67,67,67,32) edge-replicated, then pack the
    4x4 (d,h)-neighborhood redundantly in fp16:
        Q[d, h, w, c, i, j] = Gpad[d+i, h+j, w, c]   (fp16, ~281 MB)
    so each query's full 4x4x4x32 neighborhood is ONE 4KB contiguous run
    (4 consecutive 512-element rows), base=(sd*64+sh)*67+sw.
  * Device (per core, 16384 queries = 128 blocks of 128):
      stage 1: cubic basis weights + base indices; PE-transpose to
               query-on-partition layout. The w-axis weights are normalized
               by ww1 (always >= 1/6, so ratios stay bounded <= 4):
               r_k = ww_k/ww1, and ww1 is folded into the wd*wh products.
               This removes the k=1 multiply entirely: the w-contraction is
               A4 = r0*G0 + G1 + r2*G2 + r3*G3.
      main loop, one compute group = 4 blocks, gathered into one
      [128, 4, 2048] tile by 4 single-index indirect DMAs (multi-index
      gathers crash HW), descriptor-gen issued 3 groups ahead:
        - r0*G0 on DVE (TensorScalar, 4x mode); r2*G2, r3*G3 on Act.
        - batched over 4 blocks on DVE (2x mode): t01 = T0 + G1(raw);
          t23 = T2 + T3; A4 = t01 + t23; P4 = A4 * (wd*wh*ww1) broadcast.
        - (i,j) add-tree: R8 (DVE), R4 (GpSimd), R2, final f16 out (DVE).
        - one 1KB/partition output DMA per 16 blocks.
  * Host: transpose per-core outputs to [block, query, c] order and concat.
"""
import sys

for _p in ("/opt/trn_rl_repo",):
    if _p not in sys.path:
        sys.path.insert(0, _p)

import numpy as np

N_CORES = 8
B_GLOBAL = 131072
B_LOCAL = B_GLOBAL // N_CORES          # 16384
NBLK = B_LOCAL // 128                  # 128 blocks of 128 queries
GD = GH = GW = 64                      # grid spatial dims
GC = 32                                # channels
QROWS = GD * GH * (GW + 3)             # 64*64*67 = 274432
QROWLEN = 4 * 4 * GC                   # 512 elements per (d,h,w) row
GLEN = 4 * QROWLEN                     # 2048 elements gathered per query
NCOMP = 4                              # blocks per compute group
NOUT = 16                              # blocks per output DMA
LOOKG = 3                              # gather groups in flight ahead

_nc_cache = None


def _build_nc(body_reps: int = 1):
    """Build + compile the per-core Bass program (identical on all cores)."""
    from concourse import bacc, mybir
    from concourse.bass import IndirectOffsetOnAxis
    from concourse.tile import TileContext
    from concourse.masks import make_identity

    f32, f16, i32 = mybir.dt.float32, mybir.dt.float16, mybir.dt.int32
    Alu = mybir.AluOpType
    Act = mybir.ActivationFunctionType
    P = 128

    nc = bacc.Bacc("TRN2", target_bir_lowering=False, debug=False,
                   num_devices=N_CORES)
    u_t = nc.dram_tensor("u", [B_LOCAL, 3], f32, kind="ExternalInput")
    q_t = nc.dram_tensor("q", [QROWS, QROWLEN], f16, kind="ExternalInput")
    o_t = nc.dram_tensor("o", [P, NBLK * GC], f16, kind="ExternalOutput")

    with TileContext(nc) as tc:
        with (
            tc.tile_pool(name="persist", bufs=1) as pp,
            tc.tile_pool(name="stage1", bufs=1) as s1,
            tc.tile_pool(name="psum", bufs=2, space="PSUM") as psum,
            tc.tile_pool(name="g", bufs=LOOKG + 2) as gp,
            tc.tile_pool(name="t0", bufs=2) as t0p,
            tc.tile_pool(name="t2", bufs=2) as t2p,
            tc.tile_pool(name="t3", bufs=2) as t3p,
            tc.tile_pool(name="t01", bufs=2) as t01p,
            tc.tile_pool(name="t23", bufs=2) as t23p,
            tc.tile_pool(name="p4", bufs=2) as p4p,
            tc.tile_pool(name="rt", bufs=2) as rt,
            tc.tile_pool(name="o", bufs=2) as op_,
        ):
            # ---------- stage 1: weights + indices (block layout) ----------
            # U[p, n, a] = u[p*128 + n, a]; per-partition 1536B contiguous.
            U = s1.tile([P, 384], f32)
            nc.sync.dma_start(
                out=U[:, :], in_=u_t[:, :].rearrange("(p n) c -> p (n c)", p=P))
            X = s1.tile([P, 384], f32)
            nc.vector.tensor_scalar(X[:, :], U[:, :], float(GD - 1), None, Alu.mult)
            # floor via round-to-nearest cast + correction
            Si = s1.tile([P, 384], i32)
            nc.vector.tensor_copy(out=Si[:, :], in_=X[:, :])
            Sf = s1.tile([P, 384], f32)
            nc.vector.tensor_copy(out=Sf[:, :], in_=Si[:, :])
            D = s1.tile([P, 384], f32)
            nc.vector.tensor_tensor(out=D[:, :], in0=X[:, :], in1=Sf[:, :],
                                    op=Alu.subtract)
            M = s1.tile([P, 384], f32)
            nc.vector.tensor_scalar(M[:, :], D[:, :], 0.0, None, Alu.is_lt)
            S = s1.tile([P, 384], f32)
            nc.vector.tensor_tensor(out=S[:, :], in0=Sf[:, :], in1=M[:, :],
                                    op=Alu.subtract)
            T = s1.tile([P, 384], f32)
            nc.vector.tensor_tensor(out=T[:, :], in0=X[:, :], in1=S[:, :],
                                    op=Alu.subtract)

            S3 = S[:, :].rearrange("p (n c) -> p n c", c=3)
            # base = (sd*64 + sh)*67 + sw
            Bse = s1.tile([P, 128], f32)
            nc.vector.scalar_tensor_tensor(
                out=Bse[:, :], in0=S3[:, :, 0], scalar=float(GH),
                in1=S3[:, :, 1], op0=Alu.mult, op1=Alu.add)
            nc.vector.scalar_tensor_tensor(
                out=Bse[:, :], in0=Bse[:, :], scalar=float(GW + 3),
                in1=S3[:, :, 2], op0=Alu.mult, op1=Alu.add)

            # cubic basis weights on [128, 384] (all 3 axes at once)
            T2_ = s1.tile([P, 384], f32)
            nc.vector.tensor_tensor(out=T2_[:, :], in0=T[:, :], in1=T[:, :],
                                    op=Alu.mult)
            T3_ = s1.tile([P, 384], f32)
            nc.vector.tensor_tensor(out=T3_[:, :], in0=T2_[:, :], in1=T[:, :],
                                    op=Alu.mult)
            sixth = 1.0 / 6.0
            W0 = s1.tile([P, 384], f32)
            nc.vector.tensor_scalar(W0[:, :], T3_[:, :], -sixth, None, Alu.mult)
            nc.vector.scalar_tensor_tensor(out=W0[:, :], in0=T2_[:, :], scalar=0.5,
                                           in1=W0[:, :], op0=Alu.mult, op1=Alu.add)
            nc.vector.scalar_tensor_tensor(out=W0[:, :], in0=T[:, :], scalar=-0.5,
                                           in1=W0[:, :], op0=Alu.mult, op1=Alu.add)
            nc.vector.tensor_scalar(W0[:, :], W0[:, :], sixth, None, Alu.add)
            W1 = s1.tile([P, 384], f32)
            nc.vector.tensor_scalar(W1[:, :], T3_[:, :], 0.5, None, Alu.mult)
            nc.vector.scalar_tensor_tensor(out=W1[:, :], in0=T2_[:, :], scalar=-1.0,
                                           in1=W1[:, :], op0=Alu.mult, op1=Alu.add)
            nc.vector.tensor_scalar(W1[:, :], W1[:, :], 2.0 / 3.0, None, Alu.add)
            W3 = s1.tile([P, 384], f32)
            nc.vector.tensor_scalar(W3[:, :], T3_[:, :], sixth, None, Alu.mult)
            # w2 = 1 - w0 - w1 - w3  (partition of unity)
            W2 = s1.tile([P, 384], f32)
            nc.vector.tensor_tensor(out=W2[:, :], in0=W0[:, :], in1=W1[:, :],
                                    op=Alu.add)
            nc.vector.tensor_tensor(out=W2[:, :], in0=W2[:, :], in1=W3[:, :],
                                    op=Alu.add)
            nc.vector.tensor_scalar(W2[:, :], W2[:, :], -1.0, 1.0,
                                    Alu.mult, Alu.add)

            # ---------- transposes to query-on-partition layout ----------
            ident = pp.tile([P, P], f32)
            make_identity(nc, ident[:, :])

            TD = pp.tile([P, 512], f32)   # wd_i  at cols i*128 + b
            TH = pp.tile([P, 512], f32)   # wh_j  at cols j*128 + b
            TW = pp.tile([P, 512], f32)   # ww_k  at cols k*128 + b
            FB = pp.tile([P, 128], f32)   # base  [query, block]
            Ws = [W0, W1, W2, W3]

            def transpose_into(dst_ap, src_ap):
                pt = psum.tile([P, P], f32, space="PSUM")
                nc.tensor.transpose(out=pt[:, :], in_=src_ap, identity=ident[:, :])
                nc.vector.tensor_copy(out=dst_ap, in_=pt[:, :])

            # bases first: the first gathers only need IdxI
            transpose_into(FB[:, :], Bse[:, :])
            IdxI = pp.tile([P, 128], i32)
            nc.vector.tensor_copy(out=IdxI[:, :], in_=FB[:, :])

            for a, Tt in ((0, TD), (1, TH), (2, TW)):
                for i in range(4):
                    w3v = Ws[i][:, :].rearrange("p (n c) -> p n c", c=3)
                    transpose_into(Tt[:, i * 128:(i + 1) * 128], w3v[:, :, a])

            # normalize w-axis weights by ww1 (always >= 1/6): r_k = ww_k/ww1;
            # fold ww1 into the wd side so the wd*wh products carry it.
            W1t = pp.tile([P, 128], f32)
            nc.vector.tensor_copy(out=W1t[:, :], in_=TW[:, 128:256])
            RW1 = pp.tile([P, 128], f32)
            nc.vector.reciprocal(out=RW1[:, :], in_=W1t[:, :])
            rwb = (RW1[:, :].rearrange("p (x b) -> p x b", x=1)
                   .to_broadcast([P, 4, P]))
            TWv = TW[:, :].rearrange("p (k b) -> p k b", k=4)
            nc.vector.tensor_tensor(out=TWv[:, :, :], in0=TWv[:, :, :], in1=rwb,
                                    op=Alu.mult)
            w1tb = (W1t[:, :].rearrange("p (x b) -> p x b", x=1)
                    .to_broadcast([P, 4, P]))
            TDv = TD[:, :].rearrange("p (i b) -> p i b", i=4)
            nc.vector.tensor_tensor(out=TDv[:, :, :], in0=TDv[:, :, :], in1=w1tb,
                                    op=Alu.mult)

            # WDHt[q, b*16 + (i*4+j)] = wd_i*ww1 * wh_j   (fp16, b-major)
            WDHt = pp.tile([P, NBLK * 16], f16)
            wv = WDHt[:, :].rearrange("p (b ij) -> p b ij", ij=16)
            for i in range(4):
                for j in range(4):
                    nc.vector.tensor_tensor(
                        out=wv[:, :, i * 4 + j],
                        in0=TD[:, i * 128:(i + 1) * 128],
                        in1=TH[:, j * 128:(j + 1) * 128],
                        op=Alu.mult)

            # ---------- main loop (per-group pipeline) ----------
            NGRP = NBLK // NCOMP          # 32 groups of 4 blocks
            state = {}

            def emit_gather(n):
                """4 single-index gathers into one [P, 4, 2048] tile."""
                G = gp.tile([P, NCOMP, GLEN], f16, name="G")
                for j in range(NCOMP):
                    b = n * NCOMP + j
                    nc.gpsimd.indirect_dma_start(
                        out=G[:, j, :],
                        out_offset=None,
                        in_=q_t[:, :],
                        in_offset=IndirectOffsetOnAxis(
                            ap=IdxI[:, b:b + 1], axis=0),
                    )
                state[("G", n)] = G

            def emit_group(n, rep):
                G = state.pop(("G", n))
                b0 = n * NCOMP
                T0g = t0p.tile([P, NCOMP, QROWLEN], f16)
                T2g = t2p.tile([P, NCOMP, QROWLEN], f16)
                T3g = t3p.tile([P, NCOMP, QROWLEN], f16)
                for blk in range(NCOMP):
                    b = b0 + blk
                    Gb = G[:, blk, :]
                    # k0 on DVE (TensorScalar, 4x mode)
                    nc.vector.tensor_scalar(
                        T0g[:, blk, :], Gb[:, 0:QROWLEN],
                        TW[:, b:b + 1], None, Alu.mult)
                    # k2, k3 on Act
                    nc.scalar.activation(
                        T2g[:, blk, :], Gb[:, 2 * QROWLEN:3 * QROWLEN],
                        Act.Copy, bias=0.0, scale=TW[:, 256 + b:256 + b + 1])
                    nc.scalar.activation(
                        T3g[:, blk, :], Gb[:, 3 * QROWLEN:4 * QROWLEN],
                        Act.Copy, bias=0.0, scale=TW[:, 384 + b:384 + b + 1])
                # batched over the 4 blocks (DVE, 2x mode); k1 needs no mult
                G1 = G[:, :, QROWLEN:2 * QROWLEN]
                t01 = t01p.tile([P, NCOMP, QROWLEN], f16)
                nc.vector.tensor_tensor(out=t01[:, :, :], in0=T0g[:, :, :],
                                        in1=G1, op=Alu.add)
                t23 = t23p.tile([P, NCOMP, QROWLEN], f16)
                nc.vector.tensor_tensor(out=t23[:, :, :], in0=T2g[:, :, :],
                                        in1=T3g[:, :, :], op=Alu.add)
                nc.vector.tensor_tensor(out=t01[:, :, :], in0=t01[:, :, :],
                                        in1=t23[:, :, :], op=Alu.add)
                # multiply by wd*wh*ww1, broadcast over channels
                A4v = t01[:, :, :].rearrange("p blk (c ij) -> p blk c ij", ij=16)
                wb = (WDHt[:, b0 * 16:(b0 + NCOMP) * 16]
                      .rearrange("p (blk ij) -> p blk ij", ij=16)
                      .rearrange("p blk (x ij) -> p blk x ij", x=1)
                      .to_broadcast([P, NCOMP, GC, 16]))
                P4 = p4p.tile([P, NCOMP, GC, 16], f16)
                nc.vector.tensor_tensor(out=P4[:, :, :, :], in0=A4v[:, :, :, :],
                                        in1=wb, op=Alu.mult)
                # add-tree over the 16 (i,j) slots
                R8 = rt.tile([P, NCOMP, GC, 8], f16)
                nc.vector.tensor_tensor(out=R8[:, :, :, :], in0=P4[:, :, :, 0:8],
                                        in1=P4[:, :, :, 8:16], op=Alu.add)
                R4 = rt.tile([P, NCOMP, GC, 4], f16)
                nc.gpsimd.tensor_tensor(out=R4[:, :, :, :], in0=R8[:, :, :, 0:4],
                                        in1=R8[:, :, :, 4:8], op=Alu.add)
                R2 = rt.tile([P, NCOMP, GC, 2], f16)
                nc.vector.tensor_tensor(out=R2[:, :, :, :], in0=R4[:, :, :, 0:2],
                                        in1=R4[:, :, :, 2:4], op=Alu.add)
                if n % (NOUT // NCOMP) == 0:
                    state["O"] = op_.tile([P, NOUT // NCOMP, NCOMP, GC], f16,
                                          name="Oacc")
                O = state["O"]
                nc.vector.tensor_tensor(
                    out=O[:, n % (NOUT // NCOMP), :, :], in0=R2[:, :, :, 0],
                    in1=R2[:, :, :, 1], op=Alu.add)
                if n % (NOUT // NCOMP) == (NOUT // NCOMP) - 1 and rep == 0:
                    gg = n // (NOUT // NCOMP)
                    nc.sync.dma_start(
                        out=o_t[:, gg * NOUT * GC:(gg + 1) * NOUT * GC],
                        in_=O[:, :, :, :].rearrange("p a b c -> p (a b c)"))

            # body_reps > 1 repeats the full main loop for HW timing
            # amplification; only rep 0 writes outputs (others overwritten).
            for rep in range(body_reps):
                for n in range(LOOKG):
                    emit_gather(n)
                for n in range(NGRP):
                    if n + LOOKG < NGRP:
                        emit_gather(n + LOOKG)
                    emit_group(n, rep)
    nc.compile()
    return nc


def _pack_grid(grid: np.ndarray) -> np.ndarray:
    """(64,64,64,32) -> [QROWS, QROWLEN] fp16 with
    Q[d,h,w, c,i,j] = Gpad[d+i, h+j, w, c]."""
    gp = np.pad(grid, ((1, 2), (1, 2), (1, 2), (0, 0)), mode="edge")
    win = np.lib.stride_tricks.sliding_window_view(gp, (4, 4), axis=(0, 1))
    # win: [64, 64, 67, 32, 4, 4] = (d, h, w, c, i, j); ij innermost so the
    # on-device (d,h) contraction can tree-reduce contiguous slots.
    q = np.ascontiguousarray(win, dtype=np.float16)
    return q.reshape(QROWS, QROWLEN)


def kernel(u: np.ndarray, grid: np.ndarray) -> np.ndarray:
    global _nc_cache
    from concourse.bass_utils import run_bass_kernel_spmd

    assert u.shape == (B_GLOBAL, 3) and grid.shape == (GD, GH, GW, GC)
    if _nc_cache is None:
        _nc_cache = _build_nc()
    nc = _nc_cache

    q = _pack_grid(np.asarray(grid, dtype=np.float32))
    u = np.ascontiguousarray(u, dtype=np.float32)
    in_maps = [
        {"u": u[c * B_LOCAL:(c + 1) * B_LOCAL], "q": q} for c in range(N_CORES)
    ]
    res = run_bass_kernel_spmd(nc, in_maps, core_ids=list(range(N_CORES)))
    out = np.concatenate(
        [res.results[c]["o"].reshape(128, NBLK, GC).transpose(1, 0, 2)
         .reshape(B_LOCAL, GC) for c in range(N_CORES)], axis=0)
    return np.ascontiguousarray(out, dtype=np.float32)


if __name__ == "__main__":
    # quick self-run with random inputs
    rng = np.random.default_rng(0)
    grid = rng.standard_normal((GD, GH, GW, GC), dtype=np.float32)
    u = rng.random((B_GLOBAL, 3), dtype=np.float32)
    out = kernel(u, grid)
    print("out", out.shape, out.dtype, float(np.abs(out).mean()))


# revision 17
# speedup vs baseline: 3.7409x; 3.7409x over previous
"""Tricubic B-spline grid interpolation (CubicBSplineGrid3d) on 8 Trainium2 cores.

Strategy (data-parallel over queries, per sharding hint):
  * Host: pad grid (64,64,64,32) -> (67,67,67,32) edge-replicated, then pack the
    4x4 (d,h)-neighborhood redundantly in fp16:
        Q[d, h, w, c, i, j] = Gpad[d+i, h+j, w, c]   (fp16, ~281 MB)
    so each query's full 4x4x4x32 neighborhood is ONE 4KB contiguous run
    (4 consecutive 512-element rows), base=(sd*64+sh)*67+sw.
  * Device (per core, 16384 queries = 128 blocks of 128):
      stage 1: cubic basis weights + base indices; PE-transpose to
               query-on-partition layout. The w-axis weights are normalized
               by ww1 (always >= 1/6, so ratios stay bounded <= 4):
               r_k = ww_k/ww1, and ww1 is folded into the wd*wh products.
               This removes the k=1 multiply entirely: the w-contraction is
               A4 = r0*G0 + G1 + r2*G2 + r3*G3.
      main loop, one compute group = 4 blocks, gathered into one
      [128, 4, 2048] tile by 4 single-index indirect DMAs (multi-index
      gathers crash HW), descriptor-gen issued 3 groups ahead:
        - r0*G0 on DVE (TensorScalar, 4x mode; 1-in-4 blocks on Act);
          r2*G2, r3*G3 on Act.
        - batched over 4 blocks on DVE (2x mode): t01 = T0 + G1(raw);
          t23 = T2 + T3; A4 = t01 + t23; P4 = A4 * (wd*wh*ww1) broadcast.
        - (i,j) add-tree: R8 (DVE), R4 (GpSimd), R2, final f16 out (DVE).
        - one 1KB/partition output DMA per 16 blocks.
  * Host: transpose per-core outputs to [block, query, c] order and concat.
"""
import sys

for _p in ("/opt/trn_rl_repo",):
    if _p not in sys.path:
        sys.path.insert(0, _p)

import numpy as np

N_CORES = 8
B_GLOBAL = 131072
B_LOCAL = B_GLOBAL // N_CORES          # 16384
NBLK = B_LOCAL // 128                  # 128 blocks of 128 queries
GD = GH = GW = 64                      # grid spatial dims
GC = 32                                # channels
QROWS = GD * GH * (GW + 3)             # 64*64*67 = 274432
QROWLEN = 4 * 4 * GC                   # 512 elements per (d,h,w) row
GLEN = 4 * QROWLEN                     # 2048 elements gathered per query
NCOMP = 4                              # blocks per compute group
NOUT = 16                              # blocks per output DMA
LOOKG = 3                              # gather groups in flight ahead

_nc_cache = None


def _build_nc(body_reps: int = 1):
    """Build + compile the per-core Bass program (identical on all cores)."""
    from concourse import bacc, mybir
    from concourse.bass import IndirectOffsetOnAxis
    from concourse.tile import TileContext
    from concourse.masks import make_identity

    f32, f16, i32 = mybir.dt.float32, mybir.dt.float16, mybir.dt.int32
    Alu = mybir.AluOpType
    Act = mybir.ActivationFunctionType
    P = 128

    nc = bacc.Bacc("TRN2", target_bir_lowering=False, debug=False,
                   num_devices=N_CORES)
    u_t = nc.dram_tensor("u", [B_LOCAL, 3], f32, kind="ExternalInput")
    q_t = nc.dram_tensor("q", [QROWS, QROWLEN], f16, kind="ExternalInput")
    o_t = nc.dram_tensor("o", [P, NBLK * GC], f16, kind="ExternalOutput")

    with TileContext(nc) as tc:
        with (
            tc.tile_pool(name="persist", bufs=1) as pp,
            tc.tile_pool(name="stage1", bufs=1) as s1,
            tc.tile_pool(name="psum", bufs=2, space="PSUM") as psum,
            tc.tile_pool(name="g", bufs=LOOKG + 2) as gp,
            tc.tile_pool(name="t0", bufs=2) as t0p,
            tc.tile_pool(name="t2", bufs=2) as t2p,
            tc.tile_pool(name="t3", bufs=2) as t3p,
            tc.tile_pool(name="t01", bufs=2) as t01p,
            tc.tile_pool(name="t23", bufs=2) as t23p,
            tc.tile_pool(name="p4", bufs=2) as p4p,
            tc.tile_pool(name="rt", bufs=2) as rt,
            tc.tile_pool(name="o", bufs=2) as op_,
        ):
            # ---------- stage 1: weights + indices (block layout) ----------
            # U[p, n, a] = u[p*128 + n, a]; per-partition 1536B contiguous.
            U = s1.tile([P, 384], f32)
            nc.sync.dma_start(
                out=U[:, :], in_=u_t[:, :].rearrange("(p n) c -> p (n c)", p=P))
            X = s1.tile([P, 384], f32)
            nc.vector.tensor_scalar(X[:, :], U[:, :], float(GD - 1), None, Alu.mult)
            # floor via round-to-nearest cast + correction
            Si = s1.tile([P, 384], i32)
            nc.vector.tensor_copy(out=Si[:, :], in_=X[:, :])
            Sf = s1.tile([P, 384], f32)
            nc.vector.tensor_copy(out=Sf[:, :], in_=Si[:, :])
            D = s1.tile([P, 384], f32)
            nc.vector.tensor_tensor(out=D[:, :], in0=X[:, :], in1=Sf[:, :],
                                    op=Alu.subtract)
            M = s1.tile([P, 384], f32)
            nc.vector.tensor_scalar(M[:, :], D[:, :], 0.0, None, Alu.is_lt)
            S = s1.tile([P, 384], f32)
            nc.vector.tensor_tensor(out=S[:, :], in0=Sf[:, :], in1=M[:, :],
                                    op=Alu.subtract)
            T = s1.tile([P, 384], f32)
            nc.vector.tensor_tensor(out=T[:, :], in0=X[:, :], in1=S[:, :],
                                    op=Alu.subtract)

            S3 = S[:, :].rearrange("p (n c) -> p n c", c=3)
            # base = (sd*64 + sh)*67 + sw
            Bse = s1.tile([P, 128], f32)
            nc.vector.scalar_tensor_tensor(
                out=Bse[:, :], in0=S3[:, :, 0], scalar=float(GH),
                in1=S3[:, :, 1], op0=Alu.mult, op1=Alu.add)
            nc.vector.scalar_tensor_tensor(
                out=Bse[:, :], in0=Bse[:, :], scalar=float(GW + 3),
                in1=S3[:, :, 2], op0=Alu.mult, op1=Alu.add)

            # cubic basis weights on [128, 384] (all 3 axes at once)
            T2_ = s1.tile([P, 384], f32)
            nc.vector.tensor_tensor(out=T2_[:, :], in0=T[:, :], in1=T[:, :],
                                    op=Alu.mult)
            T3_ = s1.tile([P, 384], f32)
            nc.vector.tensor_tensor(out=T3_[:, :], in0=T2_[:, :], in1=T[:, :],
                                    op=Alu.mult)
            sixth = 1.0 / 6.0
            W0 = s1.tile([P, 384], f32)
            nc.vector.tensor_scalar(W0[:, :], T3_[:, :], -sixth, None, Alu.mult)
            nc.vector.scalar_tensor_tensor(out=W0[:, :], in0=T2_[:, :], scalar=0.5,
                                           in1=W0[:, :], op0=Alu.mult, op1=Alu.add)
            nc.vector.scalar_tensor_tensor(out=W0[:, :], in0=T[:, :], scalar=-0.5,
                                           in1=W0[:, :], op0=Alu.mult, op1=Alu.add)
            nc.vector.tensor_scalar(W0[:, :], W0[:, :], sixth, None, Alu.add)
            W1 = s1.tile([P, 384], f32)
            nc.vector.tensor_scalar(W1[:, :], T3_[:, :], 0.5, None, Alu.mult)
            nc.vector.scalar_tensor_tensor(out=W1[:, :], in0=T2_[:, :], scalar=-1.0,
                                           in1=W1[:, :], op0=Alu.mult, op1=Alu.add)
            nc.vector.tensor_scalar(W1[:, :], W1[:, :], 2.0 / 3.0, None, Alu.add)
            W3 = s1.tile([P, 384], f32)
            nc.vector.tensor_scalar(W3[:, :], T3_[:, :], sixth, None, Alu.mult)
            # w2 = 1 - w0 - w1 - w3  (partition of unity)
            W2 = s1.tile([P, 384], f32)
            nc.vector.tensor_tensor(out=W2[:, :], in0=W0[:, :], in1=W1[:, :],
                                    op=Alu.add)
            nc.vector.tensor_tensor(out=W2[:, :], in0=W2[:, :], in1=W3[:, :],
                                    op=Alu.add)
            nc.vector.tensor_scalar(W2[:, :], W2[:, :], -1.0, 1.0,
                                    Alu.mult, Alu.add)

            # ---------- transposes to query-on-partition layout ----------
            ident = pp.tile([P, P], f32)
            make_identity(nc, ident[:, :])

            TD = pp.tile([P, 512], f32)   # wd_i  at cols i*128 + b
            TH = pp.tile([P, 512], f32)   # wh_j  at cols j*128 + b
            TW = pp.tile([P, 512], f32)   # ww_k  at cols k*128 + b
            FB = pp.tile([P, 128], f32)   # base  [query, block]
            Ws = [W0, W1, W2, W3]

            def transpose_into(dst_ap, src_ap):
                pt = psum.tile([P, P], f32, space="PSUM")
                nc.tensor.transpose(out=pt[:, :], in_=src_ap, identity=ident[:, :])
                nc.vector.tensor_copy(out=dst_ap, in_=pt[:, :])

            # bases first: the first gathers only need IdxI
            transpose_into(FB[:, :], Bse[:, :])
            IdxI = pp.tile([P, 128], i32)
            nc.vector.tensor_copy(out=IdxI[:, :], in_=FB[:, :])

            for a, Tt in ((0, TD), (1, TH), (2, TW)):
                for i in range(4):
                    w3v = Ws[i][:, :].rearrange("p (n c) -> p n c", c=3)
                    transpose_into(Tt[:, i * 128:(i + 1) * 128], w3v[:, :, a])

            # normalize w-axis weights by ww1 (always >= 1/6): r_k = ww_k/ww1;
            # fold ww1 into the wd side so the wd*wh products carry it.
            W1t = pp.tile([P, 128], f32)
            nc.vector.tensor_copy(out=W1t[:, :], in_=TW[:, 128:256])
            RW1 = pp.tile([P, 128], f32)
            nc.vector.reciprocal(out=RW1[:, :], in_=W1t[:, :])
            rwb = (RW1[:, :].rearrange("p (x b) -> p x b", x=1)
                   .to_broadcast([P, 4, P]))
            TWv = TW[:, :].rearrange("p (k b) -> p k b", k=4)
            nc.vector.tensor_tensor(out=TWv[:, :, :], in0=TWv[:, :, :], in1=rwb,
                                    op=Alu.mult)
            w1tb = (W1t[:, :].rearrange("p (x b) -> p x b", x=1)
                    .to_broadcast([P, 4, P]))
            TDv = TD[:, :].rearrange("p (i b) -> p i b", i=4)
            nc.vector.tensor_tensor(out=TDv[:, :, :], in0=TDv[:, :, :], in1=w1tb,
                                    op=Alu.mult)

            # WDHt[q, b*16 + (i*4+j)] = wd_i*ww1 * wh_j   (fp16, b-major)
            WDHt = pp.tile([P, NBLK * 16], f16)
            wv = WDHt[:, :].rearrange("p (b ij) -> p b ij", ij=16)
            for i in range(4):
                for j in range(4):
                    nc.vector.tensor_tensor(
                        out=wv[:, :, i * 4 + j],
                        in0=TD[:, i * 128:(i + 1) * 128],
                        in1=TH[:, j * 128:(j + 1) * 128],
                        op=Alu.mult)

            # ---------- main loop (per-group pipeline) ----------
            NGRP = NBLK // NCOMP          # 32 groups of 4 blocks
            state = {}

            def emit_gather(n):
                """4 single-index gathers into one [P, 4, 2048] tile."""
                G = gp.tile([P, NCOMP, GLEN], f16, name="G")
                for j in range(NCOMP):
                    b = n * NCOMP + j
                    nc.gpsimd.indirect_dma_start(
                        out=G[:, j, :],
                        out_offset=None,
                        in_=q_t[:, :],
                        in_offset=IndirectOffsetOnAxis(
                            ap=IdxI[:, b:b + 1], axis=0),
                    )
                state[("G", n)] = G

            def emit_group(n, rep):
                G = state.pop(("G", n))
                b0 = n * NCOMP
                T0g = t0p.tile([P, NCOMP, QROWLEN], f16)
                T2g = t2p.tile([P, NCOMP, QROWLEN], f16)
                T3g = t3p.tile([P, NCOMP, QROWLEN], f16)
                for blk in range(NCOMP):
                    b = b0 + blk
                    Gb = G[:, blk, :]
                    # k0 on DVE (TensorScalar, 4x mode); 1-in-4 on Act to
                    # shave the DVE critical path
                    if blk == 0:
                        nc.scalar.activation(
                            T0g[:, blk, :], Gb[:, 0:QROWLEN],
                            Act.Copy, bias=0.0, scale=TW[:, b:b + 1])
                    else:
                        nc.vector.tensor_scalar(
                            T0g[:, blk, :], Gb[:, 0:QROWLEN],
                            TW[:, b:b + 1], None, Alu.mult)
                    # k2, k3 on Act
                    nc.scalar.activation(
                        T2g[:, blk, :], Gb[:, 2 * QROWLEN:3 * QROWLEN],
                        Act.Copy, bias=0.0, scale=TW[:, 256 + b:256 + b + 1])
                    nc.scalar.activation(
                        T3g[:, blk, :], Gb[:, 3 * QROWLEN:4 * QROWLEN],
                        Act.Copy, bias=0.0, scale=TW[:, 384 + b:384 + b + 1])
                # batched over the 4 blocks (DVE, 2x mode); k1 needs no mult
                G1 = G[:, :, QROWLEN:2 * QROWLEN]
                t01 = t01p.tile([P, NCOMP, QROWLEN], f16)
                nc.vector.tensor_tensor(out=t01[:, :, :], in0=T0g[:, :, :],
                                        in1=G1, op=Alu.add)
                t23 = t23p.tile([P, NCOMP, QROWLEN], f16)
                nc.vector.tensor_tensor(out=t23[:, :, :], in0=T2g[:, :, :],
                                        in1=T3g[:, :, :], op=Alu.add)
                nc.vector.tensor_tensor(out=t01[:, :, :], in0=t01[:, :, :],
                                        in1=t23[:, :, :], op=Alu.add)
                # multiply by wd*wh*ww1, broadcast over channels
                A4v = t01[:, :, :].rearrange("p blk (c ij) -> p blk c ij", ij=16)
                wb = (WDHt[:, b0 * 16:(b0 + NCOMP) * 16]
                      .rearrange("p (blk ij) -> p blk ij", ij=16)
                      .rearrange("p blk (x ij) -> p blk x ij", x=1)
                      .to_broadcast([P, NCOMP, GC, 16]))
                P4 = p4p.tile([P, NCOMP, GC, 16], f16)
                nc.vector.tensor_tensor(out=P4[:, :, :, :], in0=A4v[:, :, :, :],
                                        in1=wb, op=Alu.mult)
                # add-tree over the 16 (i,j) slots
                R8 = rt.tile([P, NCOMP, GC, 8], f16)
                nc.vector.tensor_tensor(out=R8[:, :, :, :], in0=P4[:, :, :, 0:8],
                                        in1=P4[:, :, :, 8:16], op=Alu.add)
                R4 = rt.tile([P, NCOMP, GC, 4], f16)
                nc.gpsimd.tensor_tensor(out=R4[:, :, :, :], in0=R8[:, :, :, 0:4],
                                        in1=R8[:, :, :, 4:8], op=Alu.add)
                R2 = rt.tile([P, NCOMP, GC, 2], f16)
                nc.vector.tensor_tensor(out=R2[:, :, :, :], in0=R4[:, :, :, 0:2],
                                        in1=R4[:, :, :, 2:4], op=Alu.add)
                if n % (NOUT // NCOMP) == 0:
                    state["O"] = op_.tile([P, NOUT // NCOMP, NCOMP, GC], f16,
                                          name="Oacc")
                O = state["O"]
                nc.vector.tensor_tensor(
                    out=O[:, n % (NOUT // NCOMP), :, :], in0=R2[:, :, :, 0],
                    in1=R2[:, :, :, 1], op=Alu.add)
                if n % (NOUT // NCOMP) == (NOUT // NCOMP) - 1 and rep == 0:
                    gg = n // (NOUT // NCOMP)
                    nc.sync.dma_start(
                        out=o_t[:, gg * NOUT * GC:(gg + 1) * NOUT * GC],
                        in_=O[:, :, :, :].rearrange("p a b c -> p (a b c)"))

            # body_reps > 1 repeats the full main loop for HW timing
            # amplification; only rep 0 writes outputs (others overwritten).
            for rep in range(body_reps):
                for n in range(LOOKG):
                    emit_gather(n)
                for n in range(NGRP):
                    if n + LOOKG < NGRP:
                        emit_gather(n + LOOKG)
                    emit_group(n, rep)
    nc.compile()
    return nc


def _pack_grid(grid: np.ndarray) -> np.ndarray:
    """(64,64,64,32) -> [QROWS, QROWLEN] fp16 with
    Q[d,h,w, c,i,j] = Gpad[d+i, h+j, w, c]."""
    gp = np.pad(grid, ((1, 2), (1, 2), (1, 2), (0, 0)), mode="edge")
    win = np.lib.stride_tricks.sliding_window_view(gp, (4, 4), axis=(0, 1))
    # win: [64, 64, 67, 32, 4, 4] = (d, h, w, c, i, j); ij innermost so the
    # on-device (d,h) contraction can tree-reduce contiguous slots.
    q = np.ascontiguousarray(win, dtype=np.float16)
    return q.reshape(QROWS, QROWLEN)


def kernel(u: np.ndarray, grid: np.ndarray) -> np.ndarray:
    global _nc_cache
    from concourse.bass_utils import run_bass_kernel_spmd

    assert u.shape == (B_GLOBAL, 3) and grid.shape == (GD, GH, GW, GC)
    if _nc_cache is None:
        _nc_cache = _build_nc()
    nc = _nc_cache

    q = _pack_grid(np.asarray(grid, dtype=np.float32))
    u = np.ascontiguousarray(u, dtype=np.float32)
    in_maps = [
        {"u": u[c * B_LOCAL:(c + 1) * B_LOCAL], "q": q} for c in range(N_CORES)
    ]
    res = run_bass_kernel_spmd(nc, in_maps, core_ids=list(range(N_CORES)))
    out = np.concatenate(
        [res.results[c]["o"].reshape(128, NBLK, GC).transpose(1, 0, 2)
         .reshape(B_LOCAL, GC) for c in range(N_CORES)], axis=0)
    return np.ascontiguousarray(out, dtype=np.float32)


if __name__ == "__main__":
    # quick self-run with random inputs
    rng = np.random.default_rng(0)
    grid = rng.standard_normal((GD, GH, GW, GC), dtype=np.float32)
    u = rng.random((B_GLOBAL, 3), dtype=np.float32)
    out = kernel(u, grid)
    print("out", out.shape, out.dtype, float(np.abs(out).mean()))
